# revision 48
# baseline (speedup 1.0000x reference)
"""Bass/Trainium2 kernel for nn_BoundaryLoss (8-core data-parallel), v2.

loss = mean( w * ce ) over (B=16, H=360, W=640) pixels, where
  ce = logsumexp_c(pred) - pred[target]            (C=7)
  w  = 10 if the 17-tap ellipse window around the pixel is NOT constant
       else 1 (cv2 border-replicate == border-ignoring for max/min).

Per core: 2 images, 6 row-groups of R=120 rows, software-pipelined:
  loads: target pre-encoded on host as phi = 2^t in fp16 (exact bit
         re-encoding (t+15)<<10); pred pre-cast to fp16.  t-tiles are
         interleaved with P-tiles so DVE mask work fills the DMA ramp.
  masks: MG_c = (phi == 2^c) per class (tensor_scalar @4x, from fp16)
  conv:  D = conv'(phi) via banded-lhsT matmuls, center weight -16
         => D == 0  <=>  window constant (phi = 2^t makes this exact);
         border rows replicated in the data so one weight set serves
         all groups.  b9 = 9*(D != 0) extracted to SBUF right away.
  MP:    masks * pred (TensorTensor @2x on DVE; some classes on gpsimd,
         the whole last group on gpsimd to shorten the tail)
  S:     sum_c exp(pred) via Act exp -> fp8e4m3 then fp8 DoubleRow
         pair-sum matmuls (0.5 cyc/row); PK = sum_c MP (fp16 identity)
  end:   lse = ln(S); ce = lse - PK; acc[g] += sum (b9+1)*ce  (one
         fused scalar_tensor_tensor accumulation per group)
Host: loss = sum(acc) / (B*H*W).

The steady state is Activation-paced (exp 3918ns + ln 718ns per group);
DMA order and per-engine work placement are tuned so exp/ln stream
nearly gapless (TimelineSim: 45451 ns vs 51044 ns baseline).
"""

import sys

for _p in ("/opt/trn_rl_repo",):
    if _p not in sys.path:
        sys.path.insert(0, _p)

import numpy as np
import ml_dtypes

import bass_rust
import concourse.bass as bass
import concourse.mybir as mybir
from concourse.tile import TileContext
from concourse import bass_utils

F32 = mybir.dt.float32
F16 = mybir.dt.float16
I16 = mybir.dt.int16
FP8 = mybir.dt.float8e4
alu = mybir.AluOpType
AF = mybir.ActivationFunctionType

B_PER_CORE = 2
H, W, C = 360, 640, 7
R = 120                    # rows per group
NG = H // R                # 3 groups per image
NIN = R + 4                # conv input rows (with halo)
WPAD = W + 4
DXS = [-2, -1, 0, 1, 2]
VERT = {0: [-2, -1, 0, 1, 2], -1: [-1, 0, 1], 1: [-1, 0, 1],
        -2: [-1, 0, 1], 2: [-1, 0, 1]}
NCOL = 64
N_POOL_MP = 3              # classes of MP multiply offloaded to gpsimd
PWID = 1024                # PSUM tile width: 4KB = exactly 2 banks
SPLITS = ((0, 512), (512, W))


def _build_consts():
    convw = np.zeros((128, 5 * R), dtype=np.float32)
    for dxi, dx in enumerate(DXS):
        for m in range(R):
            for dy in VERT[dx]:
                w = -16.0 if (dx == 0 and dy == 0) else 1.0
                convw[m + 2 + dy, dxi * R + m] += w
    idw = np.zeros((128, R), dtype=np.float16)
    for m in range(R):
        idw[m, m] = 1.0
    id2 = np.zeros((128, 2 * 128), dtype=np.float32)
    for i in range(2):
        for m in range(128):
            id2[m, i * 128 + m] = 1.0
    id1 = np.eye(128, dtype=np.float32)
    return (convw.astype(np.float16), idw,
            id2.astype(ml_dtypes.float8_e4m3),
            id1.astype(ml_dtypes.float8_e4m3))


def split_multiwait_drains(nc, max_waits=1):
    """Split >max_waits sync-waits into preceding single-wait
    EventSemaphore instructions on the same engine (walrus limit)."""
    fn = nc.m.functions[0]
    for bb in fn.blocks:
        for inst in list(bb.instructions):
            si = inst.sync_info
            if si is None or len(si.on_wait) <= max_waits:
                continue
            waits = list(si.on_wait)
            keep, extra = waits[:max_waits], waits[max_waits:]
            new_insts = []
            for k, wt in enumerate(extra):
                es = mybir.InstEventSemaphore(
                    name=f"{inst.name}-waitsplit-{k}", ins=[], outs=[])
                es.engine = inst.engine
                es.sync_info = bass_rust.SyncInfo(on_wait=[wt], on_update=[])
                nc.register_instruction(es, overwrite=True)
                new_insts.append(es)
            inst.sync_info = bass_rust.SyncInfo(
                on_wait=keep, on_update=list(si.on_update))
            pos = [i.name for i in bb.instructions].index(inst.name)
            for k, es in enumerate(new_insts):
                bb.instructions.insert(pos + k, es)


def _load_tctr(nc, pools, aps, b, gi, state):
    io, sm, pg, psD, psS, psPK, cst = pools
    pred, target, convw_sb, idw_sb, id2_sb, id1_sb, acc = aps
    r0 = gi * R
    t_ctr = pg.tile([128, W], F16, tag=f"t_ctr{b}{gi}")
    nc.sync.dma_start(out=t_ctr[:R, :], in_=target[b, r0:r0 + R, :])
    state[("tc", b, gi)] = t_ctr


def _load_tpad(nc, pools, aps, b, gi, state):
    io, sm, pg, psD, psS, psPK, cst = pools
    pred, target, convw_sb, idw_sb, id2_sb, id1_sb, acc = aps
    r0 = gi * R
    t_pad = pg.tile([128, WPAD], F16, tag=f"t_pad{b}{gi}")
    in_r0, in_r1 = max(r0 - 2, 0), min(r0 + R + 2, H)
    po = in_r0 - (r0 - 2)
    nc.sync.dma_start(out=t_pad[po:po + (in_r1 - in_r0), 2:2 + W],
                      in_=target[b, in_r0:in_r1, :])
    if po:
        nc.sync.dma_start(out=t_pad[0:po, 2:2 + W],
                          in_=target[b, 0:1, :].broadcast_to([po, W]))
    if in_r1 < r0 + R + 2:
        nb = r0 + R + 2 - in_r1
        nc.sync.dma_start(out=t_pad[NIN - nb:NIN, 2:2 + W],
                          in_=target[b, H - 1:H, :].broadcast_to([nb, W]))
    nc.gpsimd.tensor_copy(t_pad[:NIN, 0:2],
                          t_pad[:NIN, 2:3].broadcast_to([NIN, 2]))
    nc.gpsimd.tensor_copy(t_pad[:NIN, W + 2:W + 4],
                          t_pad[:NIN, W + 1:W + 2].broadcast_to([NIN, 2]))
    state[("t", b, gi)] = (t_pad, state.pop(("tc", b, gi)))


def _loads_t(nc, pools, aps, b, gi, state):
    _load_tctr(nc, pools, aps, b, gi, state)
    _load_tpad(nc, pools, aps, b, gi, state)


def _load_p(nc, pools, aps, b, gi, state, split=False):
    io, sm, pg, psD, psS, psPK, cst = pools
    pred, target, convw_sb, idw_sb, id2_sb, id1_sb, acc = aps
    r0 = gi * R
    P = io.tile([128, C * W], F16, tag="P")
    if split:
        for lo, hi in ((0, 3), (3, C)):
            nc.sync.dma_start(
                out=P[:R, lo * W:hi * W],
                in_=pred[b, lo:hi, r0:r0 + R, :].rearrange("c r w -> r c w"))
    else:
        nc.sync.dma_start(
            out=P[:R, :],
            in_=pred[b, :, r0:r0 + R, :].rearrange("c r w -> r c w"))
    state[("p", b, gi)] = (P, split)


def _early(nc, pools, aps, b, gi, state):
    """masks (only t_ctr-dependent) for group (b, gi)."""
    io, sm, pg, psD, psS, psPK, cst = pools
    pred, target, convw_sb, idw_sb, id2_sb, id1_sb, acc = aps
    t_pad, t_ctr = state[("t", b, gi)]
    MG = pg.tile([128, C * W], F16, tag=f"MG{b}{gi}")
    for c in range(C):
        nc.vector.tensor_scalar(out=MG[:R, c * W:(c + 1) * W],
                                in0=t_ctr[:R, :], scalar1=float(2 ** c),
                                scalar2=None, op0=alu.is_equal)
    state[("e", b, gi)] = MG


def _conv(nc, pools, aps, b, gi, state):
    io, sm, pg, psD, psS, psPK, cst = pools
    pred, target, convw_sb, idw_sb, id2_sb, id1_sb, acc = aps
    t_pad, t_ctr = state.pop(("t", b, gi))
    phif = t_pad[:NIN, :]
    D_ps = psD.tile([128, PWID], F32, tag="D")
    for dxi, dx in enumerate(DXS):
        lhsT = convw_sb[:NIN, dxi * R:dxi * R + R]
        st, sp = (dxi == 0), (dxi == 4)
        for (c0, c1) in SPLITS:
            nc.tensor.matmul(D_ps[:R, c0:c1], lhsT,
                             phif[:, 2 + dx + c0:2 + dx + c1],
                             start=st, stop=sp)
    b9 = pg.tile([128, W], F16, tag=f"b9{b}{gi}")
    nc.vector.tensor_scalar(out=b9[:R, :], in0=D_ps[:R, 0:W], scalar1=0.0,
                            scalar2=9.0, op0=alu.not_equal, op1=alu.mult)
    state[("d", b, gi)] = b9


def _stage_a2(nc, pools, aps, b, gi, state, npool=None):
    """P-dependent work (MP multiplies + exp)."""
    io, sm, pg, psD, psS, psPK, cst = pools
    pred, target, convw_sb, idw_sb, id2_sb, id1_sb, acc = aps
    MG = state.pop(("e", b, gi))
    P, psplit = state.pop(("p", b, gi))
    b9 = state.pop(("d", b, gi))

    MP = io.tile([128, C * W], F16, tag="MP")
    if npool is None:
        npool = N_POOL_MP
    for c in range(C - npool, C):
        sl = slice(c * W, (c + 1) * W)
        nc.gpsimd.tensor_tensor(out=MP[:R, sl], in0=MG[:R, sl],
                                in1=P[:R, sl], op=alu.mult)
    nd = C - npool
    if nd:
        nc.vector.tensor_tensor(out=MP[:R, 0:nd * W], in0=MG[:R, 0:nd * W],
                                in1=P[:R, 0:nd * W], op=alu.mult)
    E8 = io.tile([128, C * W], FP8, tag="E8")
    if psplit:
        nc.scalar.activation(E8[:R, 0:3 * W], P[:R, 0:3 * W], AF.Exp)
        nc.scalar.activation(E8[:R, 3 * W:], P[:R, 3 * W:], AF.Exp)
    else:
        nc.scalar.activation(E8[:R, :], P[:R, :], AF.Exp)
    state[(b, gi)] = (b9, MP, E8)


def _stage_b(nc, pools, aps, b, gi, state):
    """Class-sum matmuls: S (fp8 DR) and PK (fp16 identity)."""
    io, sm, pg, psD, psS, psPK, cst = pools
    pred, target, convw_sb, idw_sb, id2_sb, id1_sb, acc = aps
    b9, MP, E8 = state[(b, gi)]

    S_ps = psS.tile([128, PWID], F32, tag="S")
    lhsT2 = id2_sb[:R, :].rearrange("p (k m) -> p k m", k=2)
    for (c0, c1) in SPLITS:
        out = S_ps[:, c0:c1]
        for ci in range(3):
            c = 2 * ci
            rhs = E8[:R, c * W:(c + 2) * W].rearrange(
                "p (k w) -> p k w", k=2)[:, :, c0:c1]
            nc.tensor.matmul(out, lhsT2, rhs, start=(ci == 0), stop=False,
                             perf_mode=mybir.MatmulPerfMode.DoubleRow,
                             skip_group_check=True)
        nc.tensor.matmul(out, id1_sb[:R, :], E8[:R, 6 * W + c0:6 * W + c1],
                         start=False, stop=True, skip_group_check=True)

    PK_ps = psPK.tile([128, PWID], F32, tag="PK")
    for (c0, c1) in SPLITS:
        for c in range(C):
            nc.tensor.matmul(PK_ps[:R, c0:c1], idw_sb[:R, :],
                             MP[:R, c * W + c0:c * W + c1],
                             start=(c == 0), stop=(c == C - 1))

    state[(b, gi)] = (b9, S_ps, PK_ps)


def _stage_c(nc, pools, aps, b, gi, state, split=False):
    """ln, ce, weighted accumulation."""
    io, sm, pg, psD, psS, psPK, cst = pools
    pred, target, convw_sb, idw_sb, id2_sb, id1_sb, acc = aps
    g = b * NG + gi
    b9, S_ps, PK_ps = state.pop((b, gi))

    lse = sm.tile([128, W], F16, tag="lse")
    ce = sm.tile([128, W], F16, tag="ce")
    junk1 = sm.tile([128, W], F16, tag="junk1")
    ranges = SPLITS if split else ((0, W),)
    for k, (c0, c1) in enumerate(ranges):
        nc.scalar.activation(lse[:R, c0:c1], S_ps[:R, c0:c1], AF.Ln)
        nc.vector.tensor_tensor(out=ce[:R, c0:c1], in0=lse[:R, c0:c1],
                                in1=PK_ps[:R, c0:c1], op=alu.subtract)
        nc.vector.scalar_tensor_tensor(out=junk1[:R, c0:c1],
                                       in0=b9[:R, c0:c1], scalar=1.0,
                                       in1=ce[:R, c0:c1],
                                       op0=alu.add, op1=alu.mult,
                                       accum_out=acc[:R, 8 * k + g:8 * k + g + 1])


def build_nc(io_bufs=3, sm_bufs=4, psd_bufs=1, pss_bufs=1, pspk_bufs=2,
             pool_mode="stack", dma_order="ifirst", conv_early=False,
             split_p0=False, np_last=5, n_npl=1, split_c_last=False):
    nc = bass.Bass()
    pred = nc.dram_tensor("pred", [B_PER_CORE, C, H, W], F16,
                          kind="ExternalInput")
    target = nc.dram_tensor("target", [B_PER_CORE, H, W], F16,
                            kind="ExternalInput")
    convw = nc.dram_tensor("convw", [128, 5 * R], F16, kind="ExternalInput")
    idw = nc.dram_tensor("idw", [128, R], F16, kind="ExternalInput")
    id2 = nc.dram_tensor("id2", [128, 2 * 128], FP8, kind="ExternalInput")
    id1 = nc.dram_tensor("id1", [128, 128], FP8, kind="ExternalInput")
    acc_out = nc.dram_tensor("acc", [128, NCOL], F32, kind="ExternalOutput")

    groups = [(b, gi) for b in range(B_PER_CORE) for gi in range(NG)]
    n = len(groups)

    with TileContext(nc, pool_alloc_mode=pool_mode) as tc:
        with (
            tc.tile_pool(name="io", bufs=io_bufs) as io,
            tc.tile_pool(name="sm", bufs=sm_bufs) as sm,
            tc.tile_pool(name="pg", bufs=1) as pg,
            tc.tile_pool(name="psD", bufs=psd_bufs, space="PSUM") as psD,
            tc.tile_pool(name="psS", bufs=pss_bufs, space="PSUM") as psS,
            tc.tile_pool(name="psPK", bufs=pspk_bufs, space="PSUM") as psPK,
            tc.tile_pool(name="const", bufs=1) as cst,
        ):
            pools = (io, sm, pg, psD, psS, psPK, cst)
            state = {}

            # first group's data loads go out before the const loads
            convw_sb = cst.tile([128, 5 * R], F16)
            idw_sb = cst.tile([128, R], F16)
            id2_sb = cst.tile([128, 2 * 128], FP8)
            id1_sb = cst.tile([128, 128], FP8)
            acc = cst.tile([128, NCOL], F32)
            aps = (pred.ap(), target.ap(), convw_sb, idw_sb, id2_sb,
                   id1_sb, acc)

            if dma_order == "ifirst":
                _load_tctr(nc, pools, aps, *groups[0], state)
                _load_p(nc, pools, aps, *groups[0], state, split=split_p0)
                _load_tpad(nc, pools, aps, *groups[0], state)
                for tname, ap_ in (("convw", convw), ("idw", idw),
                                   ("id2", id2), ("id1", id1)):
                    nc.sync.dma_start(out=dict(convw=convw_sb, idw=idw_sb,
                                               id2=id2_sb, id1=id1_sb)[tname][:, :],
                                      in_=ap_.ap())
                nc.vector.memset(acc[:, :], 0.0)
                for i in range(1, n):
                    _load_tctr(nc, pools, aps, *groups[i], state)
                    _load_tpad(nc, pools, aps, *groups[i], state)
                    _load_p(nc, pools, aps, *groups[i], state)
            elif dma_order == "pfirst":
                _load_tctr(nc, pools, aps, *groups[0], state)
                _load_p(nc, pools, aps, *groups[0], state)
                _load_tpad(nc, pools, aps, *groups[0], state)
                _load_p(nc, pools, aps, *groups[1], state)
                for tname, ap_ in (("convw", convw), ("idw", idw),
                                   ("id2", id2), ("id1", id1)):
                    nc.sync.dma_start(out=dict(convw=convw_sb, idw=idw_sb,
                                               id2=id2_sb, id1=id1_sb)[tname][:, :],
                                      in_=ap_.ap())
                nc.vector.memset(acc[:, :], 0.0)
                _loads_t(nc, pools, aps, *groups[1], state)
                for i in range(2, n):
                    _load_p(nc, pools, aps, *groups[i], state)
                    _loads_t(nc, pools, aps, *groups[i], state)
            else:
                for tname, ap_ in (("convw", convw), ("idw", idw),
                                   ("id2", id2), ("id1", id1)):
                    nc.sync.dma_start(out=dict(convw=convw_sb, idw=idw_sb,
                                               id2=id2_sb, id1=id1_sb)[tname][:, :],
                                      in_=ap_.ap())
                nc.vector.memset(acc[:, :], 0.0)
                _loads_t(nc, pools, aps, *groups[0], state)
                _loads_t(nc, pools, aps, *groups[1], state)
                for i in range(n):
                    _load_p(nc, pools, aps, *groups[i], state)
                    if i + 2 < n:
                        _loads_t(nc, pools, aps, *groups[i + 2], state)
            for i in range(n):
                _early(nc, pools, aps, *groups[i], state)
            if conv_early:
                for i in range(n):
                    _conv(nc, pools, aps, *groups[i], state)
            if not conv_early:
                _conv(nc, pools, aps, *groups[0], state)
            _stage_a2(nc, pools, aps, *groups[0], state)
            if not conv_early:
                _conv(nc, pools, aps, *groups[1], state)
            _stage_a2(nc, pools, aps, *groups[1], state)
            _stage_b(nc, pools, aps, *groups[0], state)
            for i in range(n):
                if i + 2 < n:
                    if not conv_early:
                        _conv(nc, pools, aps, *groups[i + 2], state)
                    _stage_a2(nc, pools, aps, *groups[i + 2], state,
                              npool=np_last if i + 2 >= n - n_npl else None)
                if i + 1 < n:
                    _stage_b(nc, pools, aps, *groups[i + 1], state)
                _stage_c(nc, pools, aps, *groups[i], state,
                         split=(split_c_last and i == n - 1))
            nc.sync.dma_start(out=acc_out.ap(), in_=acc[:, :])

    split_multiwait_drains(nc)
    return nc


_CACHED = {}


def _get_nc():
    if "nc" not in _CACHED:
        _CACHED["nc"] = build_nc()
        _CACHED["consts"] = _build_consts()
    return _CACHED["nc"], _CACHED["consts"]


def combine_acc(acc_tiles):
    s = 0.0
    for a in acc_tiles:
        s += a[:, 0:16].astype(np.float64).sum()
    return np.float32(s / (16 * H * W))


def kernel(pred, target):
    nc, (convw, idw, id2, id1) = _get_nc()
    pred16 = np.asarray(pred).astype(np.float16)
    # encode target as 2^t in fp16 (exact; bit pattern (t+15)<<10)
    target16 = ((np.asarray(target).astype(np.int16) + 15) << 10).view(np.float16)
    n_cores = 8
    in_maps = []
    for i in range(n_cores):
        in_maps.append({
            "pred": np.ascontiguousarray(pred16[2 * i:2 * i + 2]),
            "target": np.ascontiguousarray(target16[2 * i:2 * i + 2]),
            "convw": convw, "idw": idw, "id2": id2, "id1": id1,
        })
    res = bass_utils.run_bass_kernel_spmd(nc, in_maps,
                                          core_ids=list(range(n_cores)))
    return combine_acc([r["acc"] for r in res.results])


# revision 49
# speedup vs baseline: 1.0158x; 1.0158x over previous
"""Bass/Trainium2 kernel for nn_BoundaryLoss (8-core data-parallel), v2.

loss = mean( w * ce ) over (B=16, H=360, W=640) pixels, where
  ce = logsumexp_c(pred) - pred[target]            (C=7)
  w  = 10 if the 17-tap ellipse window around the pixel is NOT constant
       else 1 (cv2 border-replicate == border-ignoring for max/min).

Per core: 2 images, 6 row-groups of R=120 rows, software-pipelined:
  loads: target pre-encoded on host as phi = 2^t in fp16 (exact bit
         re-encoding (t+15)<<10); pred pre-cast to fp16.  t-tiles are
         interleaved with P-tiles so DVE mask work fills the DMA ramp.
  masks: MG_c = (phi == 2^c) per class (tensor_scalar @4x, from fp16)
  conv:  D = conv'(phi) via banded-lhsT matmuls, center weight -16
         => D == 0  <=>  window constant (phi = 2^t makes this exact);
         border rows replicated in the data so one weight set serves
         all groups.  b9 = 9*(D != 0) extracted to SBUF right away.
  MP:    masks * pred (TensorTensor @2x on DVE; some classes on gpsimd,
         the whole last group on gpsimd to shorten the tail)
  S:     sum_c exp(pred) via Act exp -> fp8e4m3 then fp8 DoubleRow
         pair-sum matmuls (0.5 cyc/row); PK = sum_c MP (fp16 identity)
  end:   lse = ln(S); ce = lse - PK; acc[g] += sum (b9+1)*ce  (one
         fused scalar_tensor_tensor accumulation per group)
Host: loss = sum(acc) / (B*H*W).

The steady state is Activation-paced (exp 3918ns + ln 718ns per group);
DMA order and per-engine work placement are tuned so exp/ln stream
nearly gapless (TimelineSim: 45451 ns vs 51044 ns baseline).
"""

import sys

for _p in ("/opt/trn_rl_repo",):
    if _p not in sys.path:
        sys.path.insert(0, _p)

import numpy as np
import ml_dtypes

import bass_rust
import concourse.bass as bass
import concourse.mybir as mybir
from concourse.tile import TileContext
from concourse import bass_utils

F32 = mybir.dt.float32
F16 = mybir.dt.float16
I16 = mybir.dt.int16
FP8 = mybir.dt.float8e4
alu = mybir.AluOpType
AF = mybir.ActivationFunctionType

B_PER_CORE = 2
H, W, C = 360, 640, 7
R = 120                    # rows per group
NG = H // R                # 3 groups per image
NIN = R + 4                # conv input rows (with halo)
WPAD = W + 4
DXS = [-2, -1, 0, 1, 2]
VERT = {0: [-2, -1, 0, 1, 2], -1: [-1, 0, 1], 1: [-1, 0, 1],
        -2: [-1, 0, 1], 2: [-1, 0, 1]}
NCOL = 64
N_POOL_MP = 3              # classes of MP multiply offloaded to gpsimd
PWID = 1024                # PSUM tile width: 4KB = exactly 2 banks
SPLITS = ((0, 512), (512, W))


def _build_consts():
    convw = np.zeros((128, 5 * R), dtype=np.float32)
    for dxi, dx in enumerate(DXS):
        for m in range(R):
            for dy in VERT[dx]:
                w = -16.0 if (dx == 0 and dy == 0) else 1.0
                convw[m + 2 + dy, dxi * R + m] += w
    idw = np.zeros((128, R), dtype=np.float16)
    for m in range(R):
        idw[m, m] = 1.0
    id2 = np.zeros((128, 2 * 128), dtype=np.float32)
    for i in range(2):
        for m in range(128):
            id2[m, i * 128 + m] = 1.0
    id1 = np.eye(128, dtype=np.float32)
    return (convw.astype(np.float16), idw,
            id2.astype(ml_dtypes.float8_e4m3),
            id1.astype(ml_dtypes.float8_e4m3))


def split_multiwait_drains(nc, max_waits=1):
    """Split >max_waits sync-waits into preceding single-wait
    EventSemaphore instructions on the same engine (walrus limit)."""
    fn = nc.m.functions[0]
    for bb in fn.blocks:
        for inst in list(bb.instructions):
            si = inst.sync_info
            if si is None or len(si.on_wait) <= max_waits:
                continue
            waits = list(si.on_wait)
            keep, extra = waits[:max_waits], waits[max_waits:]
            new_insts = []
            for k, wt in enumerate(extra):
                es = mybir.InstEventSemaphore(
                    name=f"{inst.name}-waitsplit-{k}", ins=[], outs=[])
                es.engine = inst.engine
                es.sync_info = bass_rust.SyncInfo(on_wait=[wt], on_update=[])
                nc.register_instruction(es, overwrite=True)
                new_insts.append(es)
            inst.sync_info = bass_rust.SyncInfo(
                on_wait=keep, on_update=list(si.on_update))
            pos = [i.name for i in bb.instructions].index(inst.name)
            for k, es in enumerate(new_insts):
                bb.instructions.insert(pos + k, es)


def _load_tctr(nc, pools, aps, b, gi, state):
    io, sm, pg, psD, psS, psPK, cst = pools
    pred, target, convw_sb, idw_sb, id2_sb, id1_sb, acc = aps
    r0 = gi * R
    t_ctr = pg.tile([128, W], F16, tag=f"t_ctr{b}{gi}")
    nc.sync.dma_start(out=t_ctr[:R, :], in_=target[b, r0:r0 + R, :])
    state[("tc", b, gi)] = t_ctr


def _load_tpad(nc, pools, aps, b, gi, state):
    io, sm, pg, psD, psS, psPK, cst = pools
    pred, target, convw_sb, idw_sb, id2_sb, id1_sb, acc = aps
    r0 = gi * R
    t_pad = pg.tile([128, WPAD], F16, tag=f"t_pad{b}{gi}")
    in_r0, in_r1 = max(r0 - 2, 0), min(r0 + R + 2, H)
    po = in_r0 - (r0 - 2)
    nc.sync.dma_start(out=t_pad[po:po + (in_r1 - in_r0), 2:2 + W],
                      in_=target[b, in_r0:in_r1, :])
    if po:
        nc.sync.dma_start(out=t_pad[0:po, 2:2 + W],
                          in_=target[b, 0:1, :].broadcast_to([po, W]))
    if in_r1 < r0 + R + 2:
        nb = r0 + R + 2 - in_r1
        nc.sync.dma_start(out=t_pad[NIN - nb:NIN, 2:2 + W],
                          in_=target[b, H - 1:H, :].broadcast_to([nb, W]))
    nc.gpsimd.tensor_copy(t_pad[:NIN, 0:2],
                          t_pad[:NIN, 2:3].broadcast_to([NIN, 2]))
    nc.gpsimd.tensor_copy(t_pad[:NIN, W + 2:W + 4],
                          t_pad[:NIN, W + 1:W + 2].broadcast_to([NIN, 2]))
    state[("t", b, gi)] = (t_pad, state.pop(("tc", b, gi)))


def _loads_t(nc, pools, aps, b, gi, state):
    _load_tctr(nc, pools, aps, b, gi, state)
    _load_tpad(nc, pools, aps, b, gi, state)


def _load_p(nc, pools, aps, b, gi, state, split=False):
    io, sm, pg, psD, psS, psPK, cst = pools
    pred, target, convw_sb, idw_sb, id2_sb, id1_sb, acc = aps
    r0 = gi * R
    P = io.tile([128, C * W], F16, tag="P")
    if split:
        for lo, hi in ((0, 3), (3, C)):
            nc.sync.dma_start(
                out=P[:R, lo * W:hi * W],
                in_=pred[b, lo:hi, r0:r0 + R, :].rearrange("c r w -> r c w"))
    else:
        nc.sync.dma_start(
            out=P[:R, :],
            in_=pred[b, :, r0:r0 + R, :].rearrange("c r w -> r c w"))
    state[("p", b, gi)] = (P, split)


def _early(nc, pools, aps, b, gi, state):
    """masks (only t_ctr-dependent) for group (b, gi)."""
    io, sm, pg, psD, psS, psPK, cst = pools
    pred, target, convw_sb, idw_sb, id2_sb, id1_sb, acc = aps
    t_pad, t_ctr = state[("t", b, gi)]
    MG = pg.tile([128, C * W], F16, tag=f"MG{b}{gi}")
    for c in range(C):
        nc.vector.tensor_scalar(out=MG[:R, c * W:(c + 1) * W],
                                in0=t_ctr[:R, :], scalar1=float(2 ** c),
                                scalar2=None, op0=alu.is_equal)
    state[("e", b, gi)] = MG


def _conv(nc, pools, aps, b, gi, state):
    io, sm, pg, psD, psS, psPK, cst = pools
    pred, target, convw_sb, idw_sb, id2_sb, id1_sb, acc = aps
    t_pad, t_ctr = state.pop(("t", b, gi))
    phif = t_pad[:NIN, :]
    D_ps = psD.tile([128, PWID], F32, tag="D")
    for dxi, dx in enumerate(DXS):
        lhsT = convw_sb[:NIN, dxi * R:dxi * R + R]
        st, sp = (dxi == 0), (dxi == 4)
        for (c0, c1) in SPLITS:
            nc.tensor.matmul(D_ps[:R, c0:c1], lhsT,
                             phif[:, 2 + dx + c0:2 + dx + c1],
                             start=st, stop=sp)
    b9 = pg.tile([128, W], F16, tag=f"b9{b}{gi}")
    nc.vector.tensor_scalar(out=b9[:R, :], in0=D_ps[:R, 0:W], scalar1=0.0,
                            scalar2=9.0, op0=alu.not_equal, op1=alu.mult)
    state[("d", b, gi)] = b9


def _stage_a2(nc, pools, aps, b, gi, state, npool=None):
    """P-dependent work (MP multiplies + exp)."""
    io, sm, pg, psD, psS, psPK, cst = pools
    pred, target, convw_sb, idw_sb, id2_sb, id1_sb, acc = aps
    MG = state.pop(("e", b, gi))
    P, psplit = state.pop(("p", b, gi))
    b9 = state.pop(("d", b, gi))

    MP = io.tile([128, C * W], F16, tag="MP")
    if npool is None:
        npool = N_POOL_MP
    for c in range(C - npool, C):
        sl = slice(c * W, (c + 1) * W)
        nc.gpsimd.tensor_tensor(out=MP[:R, sl], in0=MG[:R, sl],
                                in1=P[:R, sl], op=alu.mult)
    nd = C - npool
    if nd:
        nc.vector.tensor_tensor(out=MP[:R, 0:nd * W], in0=MG[:R, 0:nd * W],
                                in1=P[:R, 0:nd * W], op=alu.mult)
    E8 = io.tile([128, C * W], FP8, tag="E8")
    if psplit:
        nc.scalar.activation(E8[:R, 0:3 * W], P[:R, 0:3 * W], AF.Exp)
        nc.scalar.activation(E8[:R, 3 * W:], P[:R, 3 * W:], AF.Exp)
    else:
        nc.scalar.activation(E8[:R, :], P[:R, :], AF.Exp)
    state[(b, gi)] = (b9, MP, E8)


def _stage_b(nc, pools, aps, b, gi, state):
    """Class-sum matmuls: S (fp8 DR) and PK (fp16 identity)."""
    io, sm, pg, psD, psS, psPK, cst = pools
    pred, target, convw_sb, idw_sb, id2_sb, id1_sb, acc = aps
    b9, MP, E8 = state[(b, gi)]

    S_ps = psS.tile([128, PWID], F32, tag="S")
    lhsT2 = id2_sb[:R, :].rearrange("p (k m) -> p k m", k=2)
    for (c0, c1) in SPLITS:
        out = S_ps[:, c0:c1]
        for ci in range(3):
            c = 2 * ci
            rhs = E8[:R, c * W:(c + 2) * W].rearrange(
                "p (k w) -> p k w", k=2)[:, :, c0:c1]
            nc.tensor.matmul(out, lhsT2, rhs, start=(ci == 0), stop=False,
                             perf_mode=mybir.MatmulPerfMode.DoubleRow,
                             skip_group_check=True)
        nc.tensor.matmul(out, id1_sb[:R, :], E8[:R, 6 * W + c0:6 * W + c1],
                         start=False, stop=True, skip_group_check=True)

    PK_ps = psPK.tile([128, PWID], F32, tag="PK")
    for (c0, c1) in SPLITS:
        for c in range(C):
            nc.tensor.matmul(PK_ps[:R, c0:c1], idw_sb[:R, :],
                             MP[:R, c * W + c0:c * W + c1],
                             start=(c == 0), stop=(c == C - 1))

    state[(b, gi)] = (b9, S_ps, PK_ps)


def _stage_c(nc, pools, aps, b, gi, state, split=False):
    """ln, ce, weighted accumulation."""
    io, sm, pg, psD, psS, psPK, cst = pools
    pred, target, convw_sb, idw_sb, id2_sb, id1_sb, acc = aps
    g = b * NG + gi
    b9, S_ps, PK_ps = state.pop((b, gi))

    lse = sm.tile([128, W], F16, tag="lse")
    ce = sm.tile([128, W], F16, tag="ce")
    junk1 = sm.tile([128, W], F16, tag="junk1")
    ranges = SPLITS if split else ((0, W),)
    for k, (c0, c1) in enumerate(ranges):
        nc.scalar.activation(lse[:R, c0:c1], S_ps[:R, c0:c1], AF.Ln)
        nc.vector.tensor_tensor(out=ce[:R, c0:c1], in0=lse[:R, c0:c1],
                                in1=PK_ps[:R, c0:c1], op=alu.subtract)
        nc.vector.scalar_tensor_tensor(out=junk1[:R, c0:c1],
                                       in0=b9[:R, c0:c1], scalar=1.0,
                                       in1=ce[:R, c0:c1],
                                       op0=alu.add, op1=alu.mult,
                                       accum_out=acc[:R, 8 * k + g:8 * k + g + 1])


def build_nc(io_bufs=3, sm_bufs=4, psd_bufs=1, pss_bufs=1, pspk_bufs=2,
             pool_mode="stack", dma_order="ifirst", conv_early=False,
             split_p0=False, np_last=5, n_npl=1, split_c_last=False):
    nc = bass.Bass()
    pred = nc.dram_tensor("pred", [B_PER_CORE, C, H, W], F16,
                          kind="ExternalInput")
    target = nc.dram_tensor("target", [B_PER_CORE, H, W], F16,
                            kind="ExternalInput")
    convw = nc.dram_tensor("convw", [128, 5 * R], F16, kind="ExternalInput")
    idw = nc.dram_tensor("idw", [128, R], F16, kind="ExternalInput")
    id2 = nc.dram_tensor("id2", [128, 2 * 128], FP8, kind="ExternalInput")
    id1 = nc.dram_tensor("id1", [128, 128], FP8, kind="ExternalInput")
    acc_out = nc.dram_tensor("acc", [128, NCOL], F32, kind="ExternalOutput")

    groups = [(b, gi) for b in range(B_PER_CORE) for gi in range(NG)]
    n = len(groups)

    with TileContext(nc, pool_alloc_mode=pool_mode) as tc:
        with (
            tc.tile_pool(name="io", bufs=io_bufs) as io,
            tc.tile_pool(name="sm", bufs=sm_bufs) as sm,
            tc.tile_pool(name="pg", bufs=1) as pg,
            tc.tile_pool(name="psD", bufs=psd_bufs, space="PSUM") as psD,
            tc.tile_pool(name="psS", bufs=pss_bufs, space="PSUM") as psS,
            tc.tile_pool(name="psPK", bufs=pspk_bufs, space="PSUM") as psPK,
            tc.tile_pool(name="const", bufs=1) as cst,
        ):
            pools = (io, sm, pg, psD, psS, psPK, cst)
            state = {}

            # first group's data loads go out before the const loads
            convw_sb = cst.tile([128, 5 * R], F16)
            idw_sb = cst.tile([128, R], F16)
            id2_sb = cst.tile([128, 2 * 128], FP8)
            id1_sb = cst.tile([128, 128], FP8)
            acc = cst.tile([128, NCOL], F32)
            aps = (pred.ap(), target.ap(), convw_sb, idw_sb, id2_sb,
                   id1_sb, acc)

            if dma_order == "ifirst":
                _load_p(nc, pools, aps, *groups[0], state, split=split_p0)
                _load_tctr(nc, pools, aps, *groups[0], state)
                _load_tpad(nc, pools, aps, *groups[0], state)
                # consts via the SWDGE (gpsimd) queue: keeps them out of
                # the HWDGE issue slots between the P loads
                for sb_t, ap_ in ((convw_sb, convw), (idw_sb, idw),
                                  (id2_sb, id2), (id1_sb, id1)):
                    nc.gpsimd.dma_start(out=sb_t[:, :], in_=ap_.ap())
                nc.vector.memset(acc[:, :], 0.0)
                for i in range(1, n):
                    _load_p(nc, pools, aps, *groups[i], state)
                    _load_tctr(nc, pools, aps, *groups[i], state)
                    _load_tpad(nc, pools, aps, *groups[i], state)
            elif dma_order == "pfirst":
                _load_tctr(nc, pools, aps, *groups[0], state)
                _load_p(nc, pools, aps, *groups[0], state)
                _load_tpad(nc, pools, aps, *groups[0], state)
                _load_p(nc, pools, aps, *groups[1], state)
                for tname, ap_ in (("convw", convw), ("idw", idw),
                                   ("id2", id2), ("id1", id1)):
                    nc.sync.dma_start(out=dict(convw=convw_sb, idw=idw_sb,
                                               id2=id2_sb, id1=id1_sb)[tname][:, :],
                                      in_=ap_.ap())
                nc.vector.memset(acc[:, :], 0.0)
                _loads_t(nc, pools, aps, *groups[1], state)
                for i in range(2, n):
                    _load_p(nc, pools, aps, *groups[i], state)
                    _loads_t(nc, pools, aps, *groups[i], state)
            else:
                for tname, ap_ in (("convw", convw), ("idw", idw),
                                   ("id2", id2), ("id1", id1)):
                    nc.sync.dma_start(out=dict(convw=convw_sb, idw=idw_sb,
                                               id2=id2_sb, id1=id1_sb)[tname][:, :],
                                      in_=ap_.ap())
                nc.vector.memset(acc[:, :], 0.0)
                _loads_t(nc, pools, aps, *groups[0], state)
                _loads_t(nc, pools, aps, *groups[1], state)
                for i in range(n):
                    _load_p(nc, pools, aps, *groups[i], state)
                    if i + 2 < n:
                        _loads_t(nc, pools, aps, *groups[i + 2], state)
            for i in range(n):
                _early(nc, pools, aps, *groups[i], state)
            if conv_early:
                for i in range(n):
                    _conv(nc, pools, aps, *groups[i], state)
            if not conv_early:
                _conv(nc, pools, aps, *groups[0], state)
            _stage_a2(nc, pools, aps, *groups[0], state)
            if not conv_early:
                _conv(nc, pools, aps, *groups[1], state)
            _stage_a2(nc, pools, aps, *groups[1], state)
            _stage_b(nc, pools, aps, *groups[0], state)
            for i in range(n):
                if i + 2 < n:
                    if not conv_early:
                        _conv(nc, pools, aps, *groups[i + 2], state)
                    _stage_a2(nc, pools, aps, *groups[i + 2], state,
                              npool=np_last if i + 2 >= n - n_npl else None)
                if i + 1 < n:
                    _stage_b(nc, pools, aps, *groups[i + 1], state)
                _stage_c(nc, pools, aps, *groups[i], state,
                         split=(split_c_last and i == n - 1))
            nc.sync.dma_start(out=acc_out.ap(), in_=acc[:, :])

    split_multiwait_drains(nc)
    return nc


_CACHED = {}


def _get_nc():
    if "nc" not in _CACHED:
        _CACHED["nc"] = build_nc()
        _CACHED["consts"] = _build_consts()
    return _CACHED["nc"], _CACHED["consts"]


def combine_acc(acc_tiles):
    s = 0.0
    for a in acc_tiles:
        s += a[:, 0:16].astype(np.float64).sum()
    return np.float32(s / (16 * H * W))


def kernel(pred, target):
    nc, (convw, idw, id2, id1) = _get_nc()
    pred16 = np.asarray(pred).astype(np.float16)
    # encode target as 2^t in fp16 (exact; bit pattern (t+15)<<10)
    target16 = ((np.asarray(target).astype(np.int16) + 15) << 10).view(np.float16)
    n_cores = 8
    in_maps = []
    for i in range(n_cores):
        in_maps.append({
            "pred": np.ascontiguousarray(pred16[2 * i:2 * i + 2]),
            "target": np.ascontiguousarray(target16[2 * i:2 * i + 2]),
            "convw": convw, "idw": idw, "id2": id2, "id1": id1,
        })
    res = bass_utils.run_bass_kernel_spmd(nc, in_maps,
                                          core_ids=list(range(n_cores)))
    return combine_acc([r["acc"] for r in res.results])


# revision 50
# speedup vs baseline: 1.0340x; 1.0179x over previous
"""Bass/Trainium2 kernel for nn_BoundaryLoss (8-core data-parallel), v2.

loss = mean( w * ce ) over (B=16, H=360, W=640) pixels, where
  ce = logsumexp_c(pred) - pred[target]            (C=7)
  w  = 10 if the 17-tap ellipse window around the pixel is NOT constant
       else 1 (cv2 border-replicate == border-ignoring for max/min).

Per core: 2 images, 6 row-groups of R=120 rows, software-pipelined:
  loads: target pre-encoded on host as phi = 2^t in fp16 (exact bit
         re-encoding (t+15)<<10); pred pre-cast to fp16.  t-tiles are
         interleaved with P-tiles so DVE mask work fills the DMA ramp.
  masks: MG_c = (phi == 2^c) per class (tensor_scalar @4x, from fp16)
  conv:  D = conv'(phi) via banded-lhsT matmuls, center weight -16
         => D == 0  <=>  window constant (phi = 2^t makes this exact);
         border rows replicated in the data so one weight set serves
         all groups.  b9 = 9*(D != 0) extracted to SBUF right away.
  MP:    masks * pred (TensorTensor @2x on DVE; some classes on gpsimd,
         the whole last group on gpsimd to shorten the tail)
  S:     sum_c exp(pred) via Act exp -> fp8e4m3 then fp8 DoubleRow
         pair-sum matmuls (0.5 cyc/row); PK = sum_c MP (fp16 identity)
  end:   lse = ln(S); ce = lse - PK; acc[g] += sum (b9+1)*ce  (one
         fused scalar_tensor_tensor accumulation per group)
Host: loss = sum(acc) / (B*H*W).

The steady state is Activation-paced (exp 3918ns + ln 718ns per group);
DMA order and per-engine work placement are tuned so exp/ln stream
nearly gapless (TimelineSim: 45451 ns vs 51044 ns baseline).
"""

import sys

for _p in ("/opt/trn_rl_repo",):
    if _p not in sys.path:
        sys.path.insert(0, _p)

import numpy as np
import ml_dtypes

import bass_rust
import concourse.bass as bass
import concourse.mybir as mybir
from concourse.tile import TileContext
from concourse import bass_utils

F32 = mybir.dt.float32
F16 = mybir.dt.float16
I16 = mybir.dt.int16
FP8 = mybir.dt.float8e4
alu = mybir.AluOpType
AF = mybir.ActivationFunctionType

B_PER_CORE = 2
H, W, C = 360, 640, 7
R = 120                    # rows per group
NG = H // R                # 3 groups per image
NIN = R + 4                # conv input rows (with halo)
WPAD = W + 4
DXS = [-2, -1, 0, 1, 2]
VERT = {0: [-2, -1, 0, 1, 2], -1: [-1, 0, 1], 1: [-1, 0, 1],
        -2: [-1, 0, 1], 2: [-1, 0, 1]}
NCOL = 64
N_POOL_MP = 3              # classes of MP multiply offloaded to gpsimd
PWID = 1024                # PSUM tile width: 4KB = exactly 2 banks
SPLITS = ((0, 512), (512, W))


def _build_consts():
    convw = np.zeros((128, 5 * R), dtype=np.float32)
    for dxi, dx in enumerate(DXS):
        for m in range(R):
            for dy in VERT[dx]:
                w = -16.0 if (dx == 0 and dy == 0) else 1.0
                convw[m + 2 + dy, dxi * R + m] += w
    idw = np.zeros((128, R), dtype=np.float16)
    for m in range(R):
        idw[m, m] = 1.0
    id2 = np.zeros((128, 2 * 128), dtype=np.float32)
    for i in range(2):
        for m in range(128):
            id2[m, i * 128 + m] = 1.0
    id1 = np.eye(128, dtype=np.float32)
    return (convw.astype(np.float16), idw,
            id2.astype(ml_dtypes.float8_e4m3),
            id1.astype(ml_dtypes.float8_e4m3))


def split_multiwait_drains(nc, max_waits=1):
    """Split >max_waits sync-waits into preceding single-wait
    EventSemaphore instructions on the same engine (walrus limit)."""
    fn = nc.m.functions[0]
    for bb in fn.blocks:
        for inst in list(bb.instructions):
            si = inst.sync_info
            if si is None or len(si.on_wait) <= max_waits:
                continue
            waits = list(si.on_wait)
            keep, extra = waits[:max_waits], waits[max_waits:]
            new_insts = []
            for k, wt in enumerate(extra):
                es = mybir.InstEventSemaphore(
                    name=f"{inst.name}-waitsplit-{k}", ins=[], outs=[])
                es.engine = inst.engine
                es.sync_info = bass_rust.SyncInfo(on_wait=[wt], on_update=[])
                nc.register_instruction(es, overwrite=True)
                new_insts.append(es)
            inst.sync_info = bass_rust.SyncInfo(
                on_wait=keep, on_update=list(si.on_update))
            pos = [i.name for i in bb.instructions].index(inst.name)
            for k, es in enumerate(new_insts):
                bb.instructions.insert(pos + k, es)


def _load_tctr(nc, pools, aps, b, gi, state):
    io, sm, pg, psD, psS, psPK, cst = pools
    pred, target, convw_sb, idw_sb, id2_sb, id1_sb, acc = aps
    r0 = gi * R
    t_ctr = pg.tile([128, W], F16, tag=f"t_ctr{b}{gi}")
    nc.sync.dma_start(out=t_ctr[:R, :], in_=target[b, r0:r0 + R, :])
    state[("tc", b, gi)] = t_ctr


def _load_tpad(nc, pools, aps, b, gi, state):
    io, sm, pg, psD, psS, psPK, cst = pools
    pred, target, convw_sb, idw_sb, id2_sb, id1_sb, acc = aps
    r0 = gi * R
    t_pad = pg.tile([128, WPAD], F16, tag=f"t_pad{b}{gi}")
    in_r0, in_r1 = max(r0 - 2, 0), min(r0 + R + 2, H)
    po = in_r0 - (r0 - 2)
    nc.sync.dma_start(out=t_pad[po:po + (in_r1 - in_r0), 2:2 + W],
                      in_=target[b, in_r0:in_r1, :])
    if po:
        nc.sync.dma_start(out=t_pad[0:po, 2:2 + W],
                          in_=target[b, 0:1, :].broadcast_to([po, W]))
    if in_r1 < r0 + R + 2:
        nb = r0 + R + 2 - in_r1
        nc.sync.dma_start(out=t_pad[NIN - nb:NIN, 2:2 + W],
                          in_=target[b, H - 1:H, :].broadcast_to([nb, W]))
    nc.gpsimd.tensor_copy(t_pad[:NIN, 0:2],
                          t_pad[:NIN, 2:3].broadcast_to([NIN, 2]))
    nc.gpsimd.tensor_copy(t_pad[:NIN, W + 2:W + 4],
                          t_pad[:NIN, W + 1:W + 2].broadcast_to([NIN, 2]))
    state[("t", b, gi)] = (t_pad, state.pop(("tc", b, gi)))


def _loads_t(nc, pools, aps, b, gi, state):
    _load_tctr(nc, pools, aps, b, gi, state)
    _load_tpad(nc, pools, aps, b, gi, state)


def _load_p(nc, pools, aps, b, gi, state, split=False):
    io, sm, pg, psD, psS, psPK, cst = pools
    pred, target, convw_sb, idw_sb, id2_sb, id1_sb, acc = aps
    r0 = gi * R
    P = io.tile([128, C * W], F16, tag="P")
    if split:
        for lo, hi in ((0, 3), (3, C)):
            nc.sync.dma_start(
                out=P[:R, lo * W:hi * W],
                in_=pred[b, lo:hi, r0:r0 + R, :].rearrange("c r w -> r c w"))
    else:
        nc.sync.dma_start(
            out=P[:R, :],
            in_=pred[b, :, r0:r0 + R, :].rearrange("c r w -> r c w"))
    state[("p", b, gi)] = (P, split)


def _early(nc, pools, aps, b, gi, state):
    """masks (only t_ctr-dependent) for group (b, gi)."""
    io, sm, pg, psD, psS, psPK, cst = pools
    pred, target, convw_sb, idw_sb, id2_sb, id1_sb, acc = aps
    t_pad, t_ctr = state[("t", b, gi)]
    MG = pg.tile([128, C * W], F16, tag=f"MG{b}{gi}")
    for c in range(C):
        nc.vector.tensor_scalar(out=MG[:R, c * W:(c + 1) * W],
                                in0=t_ctr[:R, :], scalar1=float(2 ** c),
                                scalar2=None, op0=alu.is_equal)
    state[("e", b, gi)] = MG


def _conv(nc, pools, aps, b, gi, state):
    io, sm, pg, psD, psS, psPK, cst = pools
    pred, target, convw_sb, idw_sb, id2_sb, id1_sb, acc = aps
    t_pad, t_ctr = state.pop(("t", b, gi))
    phif = t_pad[:NIN, :]
    D_ps = psD.tile([128, PWID], F32, tag="D")
    for dxi, dx in enumerate(DXS):
        lhsT = convw_sb[:NIN, dxi * R:dxi * R + R]
        st, sp = (dxi == 0), (dxi == 4)
        for (c0, c1) in SPLITS:
            nc.tensor.matmul(D_ps[:R, c0:c1], lhsT,
                             phif[:, 2 + dx + c0:2 + dx + c1],
                             start=st, stop=sp)
    b9 = pg.tile([128, W], F16, tag=f"b9{b}{gi}")
    nc.vector.tensor_scalar(out=b9[:R, :], in0=D_ps[:R, 0:W], scalar1=0.0,
                            scalar2=9.0, op0=alu.not_equal, op1=alu.mult)
    state[("d", b, gi)] = b9


def _stage_a2(nc, pools, aps, b, gi, state, npool=None):
    """P-dependent work (MP multiplies + exp)."""
    io, sm, pg, psD, psS, psPK, cst = pools
    pred, target, convw_sb, idw_sb, id2_sb, id1_sb, acc = aps
    MG = state.pop(("e", b, gi))
    P, psplit = state.pop(("p", b, gi))
    b9 = state.pop(("d", b, gi))

    MP = io.tile([128, C * W], F16, tag="MP")
    if npool is None:
        npool = N_POOL_MP
    for c in range(C - npool, C):
        sl = slice(c * W, (c + 1) * W)
        nc.gpsimd.tensor_tensor(out=MP[:R, sl], in0=MG[:R, sl],
                                in1=P[:R, sl], op=alu.mult)
    nd = C - npool
    if nd:
        nc.vector.tensor_tensor(out=MP[:R, 0:nd * W], in0=MG[:R, 0:nd * W],
                                in1=P[:R, 0:nd * W], op=alu.mult)
    E8 = io.tile([128, C * W], FP8, tag="E8")
    if psplit:
        nc.scalar.activation(E8[:R, 0:3 * W], P[:R, 0:3 * W], AF.Exp)
        nc.scalar.activation(E8[:R, 3 * W:], P[:R, 3 * W:], AF.Exp)
    else:
        nc.scalar.activation(E8[:R, :], P[:R, :], AF.Exp)
    state[(b, gi)] = (b9, MP, E8)


def _stage_b(nc, pools, aps, b, gi, state):
    """Class-sum matmuls: S (fp8 DR) and PK (fp16 identity)."""
    io, sm, pg, psD, psS, psPK, cst = pools
    pred, target, convw_sb, idw_sb, id2_sb, id1_sb, acc = aps
    b9, MP, E8 = state[(b, gi)]

    S_ps = psS.tile([128, PWID], F32, tag="S")
    lhsT2 = id2_sb[:R, :].rearrange("p (k m) -> p k m", k=2)
    for (c0, c1) in SPLITS:
        out = S_ps[:, c0:c1]
        for ci in range(3):
            c = 2 * ci
            rhs = E8[:R, c * W:(c + 2) * W].rearrange(
                "p (k w) -> p k w", k=2)[:, :, c0:c1]
            nc.tensor.matmul(out, lhsT2, rhs, start=(ci == 0), stop=False,
                             perf_mode=mybir.MatmulPerfMode.DoubleRow,
                             skip_group_check=True)
        nc.tensor.matmul(out, id1_sb[:R, :], E8[:R, 6 * W + c0:6 * W + c1],
                         start=False, stop=True, skip_group_check=True)

    PK_ps = psPK.tile([128, PWID], F32, tag="PK")
    for (c0, c1) in SPLITS:
        for c in range(C):
            nc.tensor.matmul(PK_ps[:R, c0:c1], idw_sb[:R, :],
                             MP[:R, c * W + c0:c * W + c1],
                             start=(c == 0), stop=(c == C - 1))

    state[(b, gi)] = (b9, S_ps, PK_ps)


def _stage_c(nc, pools, aps, b, gi, state, split=False):
    """ln, ce, weighted accumulation."""
    io, sm, pg, psD, psS, psPK, cst = pools
    pred, target, convw_sb, idw_sb, id2_sb, id1_sb, acc = aps
    g = b * NG + gi
    b9, S_ps, PK_ps = state.pop((b, gi))

    lse = sm.tile([128, W], F16, tag="lse")
    ce = sm.tile([128, W], F16, tag="ce")
    junk1 = sm.tile([128, W], F16, tag="junk1")
    ranges = SPLITS if split else ((0, W),)
    for k, (c0, c1) in enumerate(ranges):
        nc.scalar.activation(lse[:R, c0:c1], S_ps[:R, c0:c1], AF.Ln)
        nc.vector.tensor_tensor(out=ce[:R, c0:c1], in0=lse[:R, c0:c1],
                                in1=PK_ps[:R, c0:c1], op=alu.subtract)
        nc.vector.scalar_tensor_tensor(out=junk1[:R, c0:c1],
                                       in0=b9[:R, c0:c1], scalar=1.0,
                                       in1=ce[:R, c0:c1],
                                       op0=alu.add, op1=alu.mult,
                                       accum_out=acc[:R, 8 * k + g:8 * k + g + 1])


def build_nc(io_bufs=3, sm_bufs=4, psd_bufs=1, pss_bufs=1, pspk_bufs=2,
             pool_mode="stack", dma_order="ifirst", conv_early=False,
             split_p0=False, np_last=4, n_npl=1, split_c_last=False):
    nc = bass.Bass()
    pred = nc.dram_tensor("pred", [B_PER_CORE, C, H, W], F16,
                          kind="ExternalInput")
    target = nc.dram_tensor("target", [B_PER_CORE, H, W], F16,
                            kind="ExternalInput")
    convw = nc.dram_tensor("convw", [128, 5 * R], F16, kind="ExternalInput")
    idw = nc.dram_tensor("idw", [128, R], F16, kind="ExternalInput")
    id2 = nc.dram_tensor("id2", [128, 2 * 128], FP8, kind="ExternalInput")
    id1 = nc.dram_tensor("id1", [128, 128], FP8, kind="ExternalInput")
    acc_out = nc.dram_tensor("acc", [128, NCOL], F32, kind="ExternalOutput")

    groups = [(b, gi) for b in range(B_PER_CORE) for gi in range(NG)]
    n = len(groups)

    with TileContext(nc, pool_alloc_mode=pool_mode) as tc:
        with (
            tc.tile_pool(name="io", bufs=io_bufs) as io,
            tc.tile_pool(name="sm", bufs=sm_bufs) as sm,
            tc.tile_pool(name="pg", bufs=1) as pg,
            tc.tile_pool(name="psD", bufs=psd_bufs, space="PSUM") as psD,
            tc.tile_pool(name="psS", bufs=pss_bufs, space="PSUM") as psS,
            tc.tile_pool(name="psPK", bufs=pspk_bufs, space="PSUM") as psPK,
            tc.tile_pool(name="const", bufs=1) as cst,
        ):
            pools = (io, sm, pg, psD, psS, psPK, cst)
            state = {}

            # first group's data loads go out before the const loads
            convw_sb = cst.tile([128, 5 * R], F16)
            idw_sb = cst.tile([128, R], F16)
            id2_sb = cst.tile([128, 2 * 128], FP8)
            id1_sb = cst.tile([128, 128], FP8)
            acc = cst.tile([128, NCOL], F32)
            aps = (pred.ap(), target.ap(), convw_sb, idw_sb, id2_sb,
                   id1_sb, acc)

            if dma_order == "ifirst":
                _load_p(nc, pools, aps, *groups[0], state, split=split_p0)
                _load_tctr(nc, pools, aps, *groups[0], state)
                _load_tpad(nc, pools, aps, *groups[0], state)
                # consts via the SWDGE (gpsimd) queue: keeps them out of
                # the HWDGE issue slots between the P loads
                for sb_t, ap_ in ((convw_sb, convw), (idw_sb, idw),
                                  (id2_sb, id2), (id1_sb, id1)):
                    nc.gpsimd.dma_start(out=sb_t[:, :], in_=ap_.ap())
                nc.vector.memset(acc[:, :], 0.0)
                for i in range(1, n):
                    _load_p(nc, pools, aps, *groups[i], state)
                    _load_tctr(nc, pools, aps, *groups[i], state)
                    _load_tpad(nc, pools, aps, *groups[i], state)
            elif dma_order == "pfirst":
                _load_tctr(nc, pools, aps, *groups[0], state)
                _load_p(nc, pools, aps, *groups[0], state)
                _load_tpad(nc, pools, aps, *groups[0], state)
                _load_p(nc, pools, aps, *groups[1], state)
                for tname, ap_ in (("convw", convw), ("idw", idw),
                                   ("id2", id2), ("id1", id1)):
                    nc.sync.dma_start(out=dict(convw=convw_sb, idw=idw_sb,
                                               id2=id2_sb, id1=id1_sb)[tname][:, :],
                                      in_=ap_.ap())
                nc.vector.memset(acc[:, :], 0.0)
                _loads_t(nc, pools, aps, *groups[1], state)
                for i in range(2, n):
                    _load_p(nc, pools, aps, *groups[i], state)
                    _loads_t(nc, pools, aps, *groups[i], state)
            else:
                for tname, ap_ in (("convw", convw), ("idw", idw),
                                   ("id2", id2), ("id1", id1)):
                    nc.sync.dma_start(out=dict(convw=convw_sb, idw=idw_sb,
                                               id2=id2_sb, id1=id1_sb)[tname][:, :],
                                      in_=ap_.ap())
                nc.vector.memset(acc[:, :], 0.0)
                _loads_t(nc, pools, aps, *groups[0], state)
                _loads_t(nc, pools, aps, *groups[1], state)
                for i in range(n):
                    _load_p(nc, pools, aps, *groups[i], state)
                    if i + 2 < n:
                        _loads_t(nc, pools, aps, *groups[i + 2], state)
            for i in range(n):
                _early(nc, pools, aps, *groups[i], state)
            if conv_early:
                for i in range(n):
                    _conv(nc, pools, aps, *groups[i], state)
            if not conv_early:
                _conv(nc, pools, aps, *groups[0], state)
            _stage_a2(nc, pools, aps, *groups[0], state)
            if not conv_early:
                _conv(nc, pools, aps, *groups[1], state)
            _stage_a2(nc, pools, aps, *groups[1], state)
            _stage_b(nc, pools, aps, *groups[0], state)
            for i in range(n):
                if i + 2 < n:
                    if not conv_early:
                        _conv(nc, pools, aps, *groups[i + 2], state)
                    _stage_a2(nc, pools, aps, *groups[i + 2], state,
                              npool=np_last if i + 2 >= n - n_npl else None)
                if i + 1 < n:
                    _stage_b(nc, pools, aps, *groups[i + 1], state)
                _stage_c(nc, pools, aps, *groups[i], state,
                         split=(split_c_last and i == n - 1))
            nc.sync.dma_start(out=acc_out.ap(), in_=acc[:, :])

    split_multiwait_drains(nc)
    return nc


_CACHED = {}


def _get_nc():
    if "nc" not in _CACHED:
        _CACHED["nc"] = build_nc()
        _CACHED["consts"] = _build_consts()
    return _CACHED["nc"], _CACHED["consts"]


def combine_acc(acc_tiles):
    s = 0.0
    for a in acc_tiles:
        s += a[:, 0:16].astype(np.float64).sum()
    return np.float32(s / (16 * H * W))


def kernel(pred, target):
    nc, (convw, idw, id2, id1) = _get_nc()
    pred16 = np.asarray(pred).astype(np.float16)
    # encode target as 2^t in fp16 (exact; bit pattern (t+15)<<10)
    target16 = ((np.asarray(target).astype(np.int16) + 15) << 10).view(np.float16)
    n_cores = 8
    in_maps = []
    for i in range(n_cores):
        in_maps.append({
            "pred": np.ascontiguousarray(pred16[2 * i:2 * i + 2]),
            "target": np.ascontiguousarray(target16[2 * i:2 * i + 2]),
            "convw": convw, "idw": idw, "id2": id2, "id1": id1,
        })
    res = bass_utils.run_bass_kernel_spmd(nc, in_maps,
                                          core_ids=list(range(n_cores)))
    return combine_acc([r["acc"] for r in res.results])


# revision 51
# speedup vs baseline: 1.0489x; 1.0144x over previous
"""Bass/Trainium2 kernel for nn_BoundaryLoss (8-core data-parallel), v2.

loss = mean( w * ce ) over (B=16, H=360, W=640) pixels, where
  ce = logsumexp_c(pred) - pred[target]            (C=7)
  w  = 10 if the 17-tap ellipse window around the pixel is NOT constant
       else 1 (cv2 border-replicate == border-ignoring for max/min).

Per core: 2 images, 6 row-groups of R=120 rows, software-pipelined:
  loads: target pre-encoded on host as phi = 2^t in fp16 (exact bit
         re-encoding (t+15)<<10); pred pre-cast to fp16.  t-tiles are
         interleaved with P-tiles so DVE mask work fills the DMA ramp.
  masks: MG_c = (phi == 2^c) per class (tensor_scalar @4x, from fp16)
  conv:  D = conv'(phi) via banded-lhsT matmuls, center weight -16
         => D == 0  <=>  window constant (phi = 2^t makes this exact);
         border rows replicated in the data so one weight set serves
         all groups.  b9 = 9*(D != 0) extracted to SBUF right away.
  MP:    masks * pred (TensorTensor @2x on DVE; some classes on gpsimd,
         the whole last group on gpsimd to shorten the tail)
  S:     sum_c exp(pred) via Act exp -> fp8e4m3 then fp8 DoubleRow
         pair-sum matmuls (0.5 cyc/row); PK = sum_c MP (fp16 identity)
  end:   lse = ln(S); ce = lse - PK; acc[g] += sum (b9+1)*ce  (one
         fused scalar_tensor_tensor accumulation per group)
Host: loss = sum(acc) / (B*H*W).

The steady state is Activation-paced (exp 3918ns + ln 718ns per group);
DMA order and per-engine work placement are tuned so exp/ln stream
nearly gapless (TimelineSim: 45451 ns vs 51044 ns baseline).
"""

import sys

for _p in ("/opt/trn_rl_repo",):
    if _p not in sys.path:
        sys.path.insert(0, _p)

import numpy as np
import ml_dtypes

import bass_rust
import concourse.bass as bass
import concourse.mybir as mybir
from concourse.tile import TileContext
from concourse import bass_utils

F32 = mybir.dt.float32
F16 = mybir.dt.float16
I16 = mybir.dt.int16
FP8 = mybir.dt.float8e4
alu = mybir.AluOpType
AF = mybir.ActivationFunctionType

B_PER_CORE = 2
H, W, C = 360, 640, 7
R = 120                    # rows per group
NG = H // R                # 3 groups per image
NIN = R + 4                # conv input rows (with halo)
WPAD = W + 4
DXS = [-2, -1, 0, 1, 2]
VERT = {0: [-2, -1, 0, 1, 2], -1: [-1, 0, 1], 1: [-1, 0, 1],
        -2: [-1, 0, 1], 2: [-1, 0, 1]}
NCOL = 64
N_POOL_MP = 3              # classes of MP multiply offloaded to gpsimd
PWID = 1024                # PSUM tile width: 4KB = exactly 2 banks
SPLITS = ((0, 512), (512, W))


def _build_consts():
    convw = np.zeros((128, 5 * R), dtype=np.float32)
    for dxi, dx in enumerate(DXS):
        for m in range(R):
            for dy in VERT[dx]:
                w = -16.0 if (dx == 0 and dy == 0) else 1.0
                convw[m + 2 + dy, dxi * R + m] += w
    idw = np.zeros((128, R), dtype=np.float16)
    for m in range(R):
        idw[m, m] = 1.0
    id2 = np.zeros((128, 2 * 128), dtype=np.float32)
    for i in range(2):
        for m in range(128):
            id2[m, i * 128 + m] = 1.0
    id1 = np.eye(128, dtype=np.float32)
    return (convw.astype(np.float16), idw,
            id2.astype(ml_dtypes.float8_e4m3),
            id1.astype(ml_dtypes.float8_e4m3))


def split_multiwait_drains(nc, max_waits=1):
    """Split >max_waits sync-waits into preceding single-wait
    EventSemaphore instructions on the same engine (walrus limit)."""
    fn = nc.m.functions[0]
    for bb in fn.blocks:
        for inst in list(bb.instructions):
            si = inst.sync_info
            if si is None or len(si.on_wait) <= max_waits:
                continue
            waits = list(si.on_wait)
            keep, extra = waits[:max_waits], waits[max_waits:]
            new_insts = []
            for k, wt in enumerate(extra):
                es = mybir.InstEventSemaphore(
                    name=f"{inst.name}-waitsplit-{k}", ins=[], outs=[])
                es.engine = inst.engine
                es.sync_info = bass_rust.SyncInfo(on_wait=[wt], on_update=[])
                nc.register_instruction(es, overwrite=True)
                new_insts.append(es)
            inst.sync_info = bass_rust.SyncInfo(
                on_wait=keep, on_update=list(si.on_update))
            pos = [i.name for i in bb.instructions].index(inst.name)
            for k, es in enumerate(new_insts):
                bb.instructions.insert(pos + k, es)


def _load_tctr(nc, pools, aps, b, gi, state):
    io, sm, pg, psD, psS, psPK, cst = pools
    pred, target, convw_sb, idw_sb, id2_sb, id1_sb, acc = aps
    r0 = gi * R
    t_ctr = pg.tile([128, W], F16, tag=f"t_ctr{b}{gi}")
    nc.sync.dma_start(out=t_ctr[:R, :], in_=target[b, r0:r0 + R, :])
    state[("tc", b, gi)] = t_ctr


def _load_tpad(nc, pools, aps, b, gi, state):
    io, sm, pg, psD, psS, psPK, cst = pools
    pred, target, convw_sb, idw_sb, id2_sb, id1_sb, acc = aps
    r0 = gi * R
    t_pad = pg.tile([128, WPAD], F16, tag=f"t_pad{b}{gi}")
    in_r0, in_r1 = max(r0 - 2, 0), min(r0 + R + 2, H)
    po = in_r0 - (r0 - 2)
    nc.sync.dma_start(out=t_pad[po:po + (in_r1 - in_r0), 2:2 + W],
                      in_=target[b, in_r0:in_r1, :])
    if po:
        nc.sync.dma_start(out=t_pad[0:po, 2:2 + W],
                          in_=target[b, 0:1, :].broadcast_to([po, W]))
    if in_r1 < r0 + R + 2:
        nb = r0 + R + 2 - in_r1
        nc.sync.dma_start(out=t_pad[NIN - nb:NIN, 2:2 + W],
                          in_=target[b, H - 1:H, :].broadcast_to([nb, W]))
    nc.vector.tensor_copy(t_pad[:NIN, 0:2],
                          t_pad[:NIN, 2:3].broadcast_to([NIN, 2]))
    nc.vector.tensor_copy(t_pad[:NIN, W + 2:W + 4],
                          t_pad[:NIN, W + 1:W + 2].broadcast_to([NIN, 2]))
    state[("t", b, gi)] = (t_pad, state.pop(("tc", b, gi)))


def _loads_t(nc, pools, aps, b, gi, state):
    _load_tctr(nc, pools, aps, b, gi, state)
    _load_tpad(nc, pools, aps, b, gi, state)


def _load_p(nc, pools, aps, b, gi, state, split=False):
    io, sm, pg, psD, psS, psPK, cst = pools
    pred, target, convw_sb, idw_sb, id2_sb, id1_sb, acc = aps
    r0 = gi * R
    P = io.tile([128, C * W], F16, tag="P")
    if split:
        for lo, hi in ((0, 3), (3, C)):
            nc.sync.dma_start(
                out=P[:R, lo * W:hi * W],
                in_=pred[b, lo:hi, r0:r0 + R, :].rearrange("c r w -> r c w"))
    else:
        nc.sync.dma_start(
            out=P[:R, :],
            in_=pred[b, :, r0:r0 + R, :].rearrange("c r w -> r c w"))
    state[("p", b, gi)] = (P, split)


def _early(nc, pools, aps, b, gi, state):
    """masks (only t_ctr-dependent) for group (b, gi)."""
    io, sm, pg, psD, psS, psPK, cst = pools
    pred, target, convw_sb, idw_sb, id2_sb, id1_sb, acc = aps
    t_pad, t_ctr = state[("t", b, gi)]
    MG = pg.tile([128, C * W], F16, tag=f"MG{b}{gi}")
    for c in range(C):
        nc.vector.tensor_scalar(out=MG[:R, c * W:(c + 1) * W],
                                in0=t_ctr[:R, :], scalar1=float(2 ** c),
                                scalar2=None, op0=alu.is_equal)
    state[("e", b, gi)] = MG


def _conv(nc, pools, aps, b, gi, state):
    io, sm, pg, psD, psS, psPK, cst = pools
    pred, target, convw_sb, idw_sb, id2_sb, id1_sb, acc = aps
    t_pad, t_ctr = state.pop(("t", b, gi))
    phif = t_pad[:NIN, :]
    D_ps = psD.tile([128, PWID], F32, tag="D")
    for dxi, dx in enumerate(DXS):
        lhsT = convw_sb[:NIN, dxi * R:dxi * R + R]
        st, sp = (dxi == 0), (dxi == 4)
        for (c0, c1) in SPLITS:
            nc.tensor.matmul(D_ps[:R, c0:c1], lhsT,
                             phif[:, 2 + dx + c0:2 + dx + c1],
                             start=st, stop=sp)
    b9 = pg.tile([128, W], F16, tag=f"b9{b}{gi}")
    nc.vector.tensor_scalar(out=b9[:R, :], in0=D_ps[:R, 0:W], scalar1=0.0,
                            scalar2=9.0, op0=alu.not_equal, op1=alu.mult)
    state[("d", b, gi)] = b9


def _stage_a2(nc, pools, aps, b, gi, state, npool=None):
    """P-dependent work (MP multiplies + exp)."""
    io, sm, pg, psD, psS, psPK, cst = pools
    pred, target, convw_sb, idw_sb, id2_sb, id1_sb, acc = aps
    MG = state.pop(("e", b, gi))
    P, psplit = state.pop(("p", b, gi))
    b9 = state.pop(("d", b, gi))

    MP = io.tile([128, C * W], F16, tag="MP")
    if npool is None:
        npool = N_POOL_MP
    for c in range(C - npool, C):
        sl = slice(c * W, (c + 1) * W)
        nc.gpsimd.tensor_tensor(out=MP[:R, sl], in0=MG[:R, sl],
                                in1=P[:R, sl], op=alu.mult)
    nd = C - npool
    if nd:
        nc.vector.tensor_tensor(out=MP[:R, 0:nd * W], in0=MG[:R, 0:nd * W],
                                in1=P[:R, 0:nd * W], op=alu.mult)
    E8 = io.tile([128, C * W], FP8, tag="E8")
    if psplit:
        nc.scalar.activation(E8[:R, 0:3 * W], P[:R, 0:3 * W], AF.Exp)
        nc.scalar.activation(E8[:R, 3 * W:], P[:R, 3 * W:], AF.Exp)
    else:
        nc.scalar.activation(E8[:R, :], P[:R, :], AF.Exp)
    state[(b, gi)] = (b9, MP, E8)


def _stage_b(nc, pools, aps, b, gi, state):
    """Class-sum matmuls: S (fp8 DR) and PK (fp16 identity)."""
    io, sm, pg, psD, psS, psPK, cst = pools
    pred, target, convw_sb, idw_sb, id2_sb, id1_sb, acc = aps
    b9, MP, E8 = state[(b, gi)]

    S_ps = psS.tile([128, PWID], F32, tag="S")
    lhsT2 = id2_sb[:R, :].rearrange("p (k m) -> p k m", k=2)
    for (c0, c1) in SPLITS:
        out = S_ps[:, c0:c1]
        for ci in range(3):
            c = 2 * ci
            rhs = E8[:R, c * W:(c + 2) * W].rearrange(
                "p (k w) -> p k w", k=2)[:, :, c0:c1]
            nc.tensor.matmul(out, lhsT2, rhs, start=(ci == 0), stop=False,
                             perf_mode=mybir.MatmulPerfMode.DoubleRow,
                             skip_group_check=True)
        nc.tensor.matmul(out, id1_sb[:R, :], E8[:R, 6 * W + c0:6 * W + c1],
                         start=False, stop=True, skip_group_check=True)

    PK_ps = psPK.tile([128, PWID], F32, tag="PK")
    for (c0, c1) in SPLITS:
        for c in range(C):
            nc.tensor.matmul(PK_ps[:R, c0:c1], idw_sb[:R, :],
                             MP[:R, c * W + c0:c * W + c1],
                             start=(c == 0), stop=(c == C - 1))

    state[(b, gi)] = (b9, S_ps, PK_ps)


def _stage_c(nc, pools, aps, b, gi, state, split=False):
    """ln, ce, weighted accumulation."""
    io, sm, pg, psD, psS, psPK, cst = pools
    pred, target, convw_sb, idw_sb, id2_sb, id1_sb, acc = aps
    g = b * NG + gi
    b9, S_ps, PK_ps = state.pop((b, gi))

    lse = sm.tile([128, W], F16, tag="lse")
    ce = sm.tile([128, W], F16, tag="ce")
    junk1 = sm.tile([128, W], F16, tag="junk1")
    ranges = SPLITS if split else ((0, W),)
    for k, (c0, c1) in enumerate(ranges):
        nc.scalar.activation(lse[:R, c0:c1], S_ps[:R, c0:c1], AF.Ln)
        nc.vector.tensor_tensor(out=ce[:R, c0:c1], in0=lse[:R, c0:c1],
                                in1=PK_ps[:R, c0:c1], op=alu.subtract)
        nc.vector.scalar_tensor_tensor(out=junk1[:R, c0:c1],
                                       in0=b9[:R, c0:c1], scalar=1.0,
                                       in1=ce[:R, c0:c1],
                                       op0=alu.add, op1=alu.mult,
                                       accum_out=acc[:R, 8 * k + g:8 * k + g + 1])


def build_nc(io_bufs=3, sm_bufs=4, psd_bufs=1, pss_bufs=1, pspk_bufs=2,
             pool_mode="stack", dma_order="ifirst", conv_early=False,
             split_p0=False, np_last=4, n_npl=1, split_c_last=False):
    nc = bass.Bass()
    pred = nc.dram_tensor("pred", [B_PER_CORE, C, H, W], F16,
                          kind="ExternalInput")
    target = nc.dram_tensor("target", [B_PER_CORE, H, W], F16,
                            kind="ExternalInput")
    convw = nc.dram_tensor("convw", [128, 5 * R], F16, kind="ExternalInput")
    idw = nc.dram_tensor("idw", [128, R], F16, kind="ExternalInput")
    id2 = nc.dram_tensor("id2", [128, 2 * 128], FP8, kind="ExternalInput")
    id1 = nc.dram_tensor("id1", [128, 128], FP8, kind="ExternalInput")
    acc_out = nc.dram_tensor("acc", [128, NCOL], F32, kind="ExternalOutput")

    groups = [(b, gi) for b in range(B_PER_CORE) for gi in range(NG)]
    n = len(groups)

    with TileContext(nc, pool_alloc_mode=pool_mode) as tc:
        with (
            tc.tile_pool(name="io", bufs=io_bufs) as io,
            tc.tile_pool(name="sm", bufs=sm_bufs) as sm,
            tc.tile_pool(name="pg", bufs=1) as pg,
            tc.tile_pool(name="psD", bufs=psd_bufs, space="PSUM") as psD,
            tc.tile_pool(name="psS", bufs=pss_bufs, space="PSUM") as psS,
            tc.tile_pool(name="psPK", bufs=pspk_bufs, space="PSUM") as psPK,
            tc.tile_pool(name="const", bufs=1) as cst,
        ):
            pools = (io, sm, pg, psD, psS, psPK, cst)
            state = {}

            # first group's data loads go out before the const loads
            convw_sb = cst.tile([128, 5 * R], F16)
            idw_sb = cst.tile([128, R], F16)
            id2_sb = cst.tile([128, 2 * 128], FP8)
            id1_sb = cst.tile([128, 128], FP8)
            acc = cst.tile([128, NCOL], F32)
            aps = (pred.ap(), target.ap(), convw_sb, idw_sb, id2_sb,
                   id1_sb, acc)

            if dma_order == "ifirst":
                _load_p(nc, pools, aps, *groups[0], state, split=split_p0)
                _load_tctr(nc, pools, aps, *groups[0], state)
                _load_tpad(nc, pools, aps, *groups[0], state)
                # consts via the SWDGE (gpsimd) queue: keeps them out of
                # the HWDGE issue slots between the P loads
                for sb_t, ap_ in ((convw_sb, convw), (idw_sb, idw),
                                  (id2_sb, id2), (id1_sb, id1)):
                    nc.gpsimd.dma_start(out=sb_t[:, :], in_=ap_.ap())
                nc.vector.memset(acc[:, :], 0.0)
                for i in range(1, n):
                    _load_p(nc, pools, aps, *groups[i], state)
                    _load_tctr(nc, pools, aps, *groups[i], state)
                    _load_tpad(nc, pools, aps, *groups[i], state)
            elif dma_order == "pfirst":
                _load_tctr(nc, pools, aps, *groups[0], state)
                _load_p(nc, pools, aps, *groups[0], state)
                _load_tpad(nc, pools, aps, *groups[0], state)
                _load_p(nc, pools, aps, *groups[1], state)
                for tname, ap_ in (("convw", convw), ("idw", idw),
                                   ("id2", id2), ("id1", id1)):
                    nc.sync.dma_start(out=dict(convw=convw_sb, idw=idw_sb,
                                               id2=id2_sb, id1=id1_sb)[tname][:, :],
                                      in_=ap_.ap())
                nc.vector.memset(acc[:, :], 0.0)
                _loads_t(nc, pools, aps, *groups[1], state)
                for i in range(2, n):
                    _load_p(nc, pools, aps, *groups[i], state)
                    _loads_t(nc, pools, aps, *groups[i], state)
            else:
                for tname, ap_ in (("convw", convw), ("idw", idw),
                                   ("id2", id2), ("id1", id1)):
                    nc.sync.dma_start(out=dict(convw=convw_sb, idw=idw_sb,
                                               id2=id2_sb, id1=id1_sb)[tname][:, :],
                                      in_=ap_.ap())
                nc.vector.memset(acc[:, :], 0.0)
                _loads_t(nc, pools, aps, *groups[0], state)
                _loads_t(nc, pools, aps, *groups[1], state)
                for i in range(n):
                    _load_p(nc, pools, aps, *groups[i], state)
                    if i + 2 < n:
                        _loads_t(nc, pools, aps, *groups[i + 2], state)
            for i in range(n):
                _early(nc, pools, aps, *groups[i], state)
            if conv_early:
                for i in range(n):
                    _conv(nc, pools, aps, *groups[i], state)
            if not conv_early:
                _conv(nc, pools, aps, *groups[0], state)
            _stage_a2(nc, pools, aps, *groups[0], state)
            if not conv_early:
                _conv(nc, pools, aps, *groups[1], state)
            _stage_a2(nc, pools, aps, *groups[1], state)
            _stage_b(nc, pools, aps, *groups[0], state)
            for i in range(n):
                if i + 2 < n:
                    if not conv_early:
                        _conv(nc, pools, aps, *groups[i + 2], state)
                    _stage_a2(nc, pools, aps, *groups[i + 2], state,
                              npool=np_last if i + 2 >= n - n_npl else None)
                if i + 1 < n:
                    _stage_b(nc, pools, aps, *groups[i + 1], state)
                _stage_c(nc, pools, aps, *groups[i], state,
                         split=(split_c_last and i == n - 1))
            nc.sync.dma_start(out=acc_out.ap(), in_=acc[:, :])

    split_multiwait_drains(nc)
    return nc


_CACHED = {}


def _get_nc():
    if "nc" not in _CACHED:
        _CACHED["nc"] = build_nc()
        _CACHED["consts"] = _build_consts()
    return _CACHED["nc"], _CACHED["consts"]


def combine_acc(acc_tiles):
    s = 0.0
    for a in acc_tiles:
        s += a[:, 0:16].astype(np.float64).sum()
    return np.float32(s / (16 * H * W))


def kernel(pred, target):
    nc, (convw, idw, id2, id1) = _get_nc()
    pred16 = np.asarray(pred).astype(np.float16)
    # encode target as 2^t in fp16 (exact; bit pattern (t+15)<<10)
    target16 = ((np.asarray(target).astype(np.int16) + 15) << 10).view(np.float16)
    n_cores = 8
    in_maps = []
    for i in range(n_cores):
        in_maps.append({
            "pred": np.ascontiguousarray(pred16[2 * i:2 * i + 2]),
            "target": np.ascontiguousarray(target16[2 * i:2 * i + 2]),
            "convw": convw, "idw": idw, "id2": id2, "id1": id1,
        })
    res = bass_utils.run_bass_kernel_spmd(nc, in_maps,
                                          core_ids=list(range(n_cores)))
    return combine_acc([r["acc"] for r in res.results])
